# revision 1
# baseline (speedup 1.0000x reference)
"""Trainium2 Bass kernel for BiomechanicGATHead.

Math restructure (exact, done host-side in float64):
  reference:
    h  = gelu(x @ W1 + b1)                       [R,256]
    n0 = h @ W2 + b2                             [R,544]   (544 = 17 nodes x 32 feat)
    GAT(n, adj, Wg, bg) = gelu((softmax(adj) @ n_nodes) @ Wg + bg) + n
    out = GAT2(GAT1(n0)) @ Wc + bc               [R,17,2]

  Flattened over (node, feat), the GAT linear part is a dense 544x544 matmul
  by  M = kron(softmax(adj).T, Wg);  its bias is tile(bg, 17).
  GAT1 is fused into the preceding linear:  W2K1 = W2 @ M1.
  The +b2 bias is deferred algebraically into downstream biases so the
  residual adds can consume raw PSUM:
    t1  = gelu(h @ W2K1 + (b2@M1 + tile(bg1,17)))
    m1  = t1 + h @ W2                 ("n1 - b2")
    t2  = gelu(m1 @ M2 + (b2@M2 + tile(bg2,17)))
    m2  = t2 + m1                     ("n2 - b2")
    out = m2 @ C + (b2@C + tile(bc,17))      with C = kron(I17, Wc) [544,34]

  544 is padded to 640 = 5*128 with zero rows/cols (pads never affect the
  output because all padded weight ROWS are zero; uniform K=128 chunks keep
  the PE stream free of tile-size reconfiguration stalls).

Device layout: activations are kept transposed -- features on SBUF
partitions, rows on the free dim -- so the host pre-transposes x per shard
([128, 8192] per core) and post-transposes the output ([34, 8192] -> rows).
Matmuls run as f32r (tf32-like, 1 cycle/row for N>=256) with fp32 PSUM
accumulation.

DMA queues: input tiles + small consts on sync (HWDGE), bulk weight slabs
on scalar's queue, output stores on gpsimd's queue -- so the next tile's
input load never serializes behind stores or weight streaming.

Sharding: pure data parallel, 65536 rows split as 8192 rows x 8 cores.
"""

import numpy as np

import concourse.bass as bass
import concourse.mybir as mybir
import concourse.tile as tile
from concourse import bacc
from concourse.bass_utils import run_bass_kernel_spmd

N_CORES = 8
D, HID, NN, ND = 128, 256, 17, 32
F = NN * ND          # 544
KC = 5               # 128-chunks covering the padded feature dim
FP = KC * 128        # 640
OUTW = NN * 2        # 34
B, W = 16, 4096
ROWS = B * W         # 65536
R_CORE = ROWS // N_CORES   # 8192
TILE_N = 512
N_TILES = R_CORE // TILE_N  # 16

f32 = mybir.dt.float32
f32r = mybir.dt.float32r
GELU = mybir.ActivationFunctionType.Gelu


def _prep_constants(W1, b1, W2, b2, adj1, Wg1, bg1, adj2, Wg2, bg2, Wc, bc):
    """Fold the network into the fused layers; return device-layout arrays."""
    d = {}
    f64 = np.float64

    def softmax(a):
        a = a.astype(f64)
        e = np.exp(a - a.max(axis=-1, keepdims=True))
        return e / e.sum(axis=-1, keepdims=True)

    A1 = softmax(adj1)
    A2 = softmax(adj2)
    M1 = np.kron(A1.T, Wg1.astype(f64))          # [544, 544]
    M2 = np.kron(A2.T, Wg2.astype(f64))          # [544, 544]
    C = np.kron(np.eye(NN), Wc.astype(f64))      # [544, 34]

    W2K1 = W2.astype(f64) @ M1                   # [256, 544]
    bK1 = b2.astype(f64) @ M1 + np.tile(bg1.astype(f64), NN)   # [544]
    bG2 = b2.astype(f64) @ M2 + np.tile(bg2.astype(f64), NN)   # [544]
    bC = b2.astype(f64) @ C + np.tile(bc.astype(f64), NN)      # [34]

    def padcols(a, w):
        out = np.zeros((a.shape[0], w), f64)
        out[:, : a.shape[1]] = a
        return out

    def padrows(a, h):
        out = np.zeros((h,) + a.shape[1:], f64)
        out[: a.shape[0]] = a
        return out

    W2p = padcols(W2.astype(f64), FP)            # [256, 640]
    W2K1p = padcols(W2K1, FP)                    # [256, 640]
    M2p = padrows(padcols(M2, FP), FP)           # [640, 640]
    Cp = padrows(C, FP)                          # [640, 34]
    bK1p = padrows(bK1, FP)                      # [640]
    bG2p = padrows(bG2, FP)                      # [640]

    asf = lambda a: np.ascontiguousarray(a, dtype=np.float32)
    # SBUF layouts: partition dim first; K-chunks as middle axis.
    d["w1"] = asf(W1)                                            # [128, 256]
    d["w2"] = asf(W2p.reshape(2, 128, FP).transpose(1, 0, 2))    # [128, 2, 640]
    d["w2k1"] = asf(W2K1p.reshape(2, 128, FP).transpose(1, 0, 2))
    d["m2"] = asf(M2p.reshape(KC, 128, FP).transpose(1, 0, 2))   # [128, 5, 640]
    d["cw"] = asf(Cp.reshape(KC, 128, OUTW).transpose(1, 0, 2))  # [128, 5, 34]
    d["b1"] = asf(b1.astype(f64).reshape(2, 128).T)              # [128, 2]
    d["bk1"] = asf(bK1p.reshape(KC, 128).T)                      # [128, 5]
    d["bg2"] = asf(bG2p.reshape(KC, 128).T)                      # [128, 5]
    d["bc"] = asf(bC.reshape(OUTW, 1))                           # [34, 1]
    return d


def _build_nc():
    """Build the per-core Bass program (same NEFF on all 8 cores)."""
    nc = bacc.Bacc("TRN2", target_bir_lowering=False, debug=False)

    xT = nc.dram_tensor("xT", [D, R_CORE], f32r, kind="ExternalInput").ap()
    w1 = nc.dram_tensor("w1", [128, HID], f32r, kind="ExternalInput").ap()
    w2 = nc.dram_tensor("w2", [128, 2, FP], f32r, kind="ExternalInput").ap()
    w2k1 = nc.dram_tensor("w2k1", [128, 2, FP], f32r, kind="ExternalInput").ap()
    m2 = nc.dram_tensor("m2", [128, KC, FP], f32r, kind="ExternalInput").ap()
    cw = nc.dram_tensor("cw", [128, KC, OUTW], f32r, kind="ExternalInput").ap()
    b1 = nc.dram_tensor("b1", [128, 2], f32, kind="ExternalInput").ap()
    bk1 = nc.dram_tensor("bk1", [128, KC], f32, kind="ExternalInput").ap()
    bg2 = nc.dram_tensor("bg2", [128, KC], f32, kind="ExternalInput").ap()
    bc = nc.dram_tensor("bc", [OUTW, 1], f32, kind="ExternalInput").ap()
    outT = nc.dram_tensor("outT", [OUTW, R_CORE], f32, kind="ExternalOutput").ap()

    with tile.TileContext(nc) as tc:
        with (
            tc.tile_pool(name="consts", bufs=1) as consts,
            tc.tile_pool(name="acts", bufs=2) as acts,
            tc.tile_pool(name="xio", bufs=3) as xio,
            tc.tile_pool(name="ps", bufs=1, space=bass.MemorySpace.PSUM) as ps,
        ):
            # L1's operands first on the fast sync queue so compute starts
            # as soon as possible; bulk slabs stream on scalar's queue and
            # are consumed a few microseconds later.
            w1s = consts.tile([128, HID], f32r)
            nc.sync.dma_start(w1s, w1)
            b1s = consts.tile([128, 2], f32)
            nc.sync.dma_start(b1s, b1)

            w2k1s = consts.tile([128, 2, FP], f32r)
            nc.scalar.dma_start(w2k1s, w2k1)
            bk1s = consts.tile([128, KC], f32)
            nc.scalar.dma_start(bk1s, bk1)
            w2s = consts.tile([128, 2, FP], f32r)
            nc.scalar.dma_start(w2s, w2)
            m2s = consts.tile([128, KC, FP], f32r)
            nc.scalar.dma_start(m2s, m2)
            bg2s = consts.tile([128, KC], f32)
            nc.scalar.dma_start(bg2s, bg2)
            cws = consts.tile([128, KC, OUTW], f32r)
            nc.scalar.dma_start(cws, cw)
            bcs = consts.tile([OUTW, 1], f32)
            nc.scalar.dma_start(bcs, bc)

            def emit_l5(p_m2s, p_sl, p_t):
                po = ps.tile([OUTW, TILE_N], f32, tag="po", bufs=1, name=f"po_{p_t}")
                for k in range(KC):
                    nc.tensor.matmul(
                        po, cws[:, k, :], p_m2s[:, k, :], start=(k == 0), stop=(k == KC - 1)
                    )
                ot = xio.tile([OUTW, TILE_N], f32, tag="ot", name=f"ot_{p_t}")
                nc.vector.tensor_scalar_add(ot, po, bcs)
                nc.gpsimd.dma_start(outT[:, p_sl], ot)

            prev = None
            for t in range(N_TILES):
                sl = bass.ts(t, TILE_N)

                xt = xio.tile([D, TILE_N], f32r, tag="xt", name=f"xt_{t}")
                nc.sync.dma_start(xt, xT[:, sl])

                # L1: hT = gelu(W1.T @ xT + b1)   [2 chunks of 128]
                ph = ps.tile([128, 2, TILE_N], f32, tag="ph", bufs=1, name=f"ph_{t}")
                for c in range(2):
                    nc.tensor.matmul(
                        ph[:, c, :], w1s[:, bass.ts(c, 128)], xt, start=True, stop=True
                    )
                hs = acts.tile([128, 2, TILE_N], f32r, tag="hs")
                for c in range(2):
                    nc.scalar.activation(hs[:, c, :], ph[:, c, :], GELU, bias=b1s[:, c : c + 1])

                # L2b: t1 = gelu(h @ W2K1 + bK1)  (GAT1 fused)
                t1s = acts.tile([128, KC, TILE_N], f32, tag="t1s")
                for m in range(KC):
                    pt1 = ps.tile([128, TILE_N], f32, tag="pp", bufs=5, name=f"pt1_{t}_{m}")
                    for k in range(2):
                        nc.tensor.matmul(
                            pt1,
                            w2k1s[:, k, bass.ts(m, 128)],
                            hs[:, k, :],
                            start=(k == 0),
                            stop=(k == 1),
                        )
                    nc.scalar.activation(t1s[:, m, :], pt1, GELU, bias=bk1s[:, m : m + 1])

                # L2a: m1 = t1 + h @ W2   (residual, b2 deferred)
                m1s = acts.tile([128, KC, TILE_N], f32r, tag="m1s")
                for m in range(KC):
                    pn0 = ps.tile([128, TILE_N], f32, tag="pp", bufs=5, name=f"pn0_{t}_{m}")
                    for k in range(2):
                        nc.tensor.matmul(
                            pn0,
                            w2s[:, k, bass.ts(m, 128)],
                            hs[:, k, :],
                            start=(k == 0),
                            stop=(k == 1),
                        )
                    nc.vector.tensor_add(m1s[:, m, :], t1s[:, m, :], pn0)

                # GAT2: t2 = gelu(m1 @ M2 + bG2)
                t2s = acts.tile([128, KC, TILE_N], f32, tag="t2s")
                for m in range(KC):
                    pt2 = ps.tile([128, TILE_N], f32, tag="pp", bufs=5, name=f"pt2_{t}_{m}")
                    for k in range(KC):
                        nc.tensor.matmul(
                            pt2,
                            m2s[:, k, bass.ts(m, 128)],
                            m1s[:, k, :],
                            start=(k == 0),
                            stop=(k == KC - 1),
                        )
                    nc.scalar.activation(t2s[:, m, :], pt2, GELU, bias=bg2s[:, m : m + 1])

                # m2 = t2 + m1 (residual)
                m2s_t = acts.tile([128, KC, TILE_N], f32r, tag="m2s_t")
                for m in range(KC):
                    nc.vector.tensor_add(m2s_t[:, m, :], t2s[:, m, :], m1s[:, m, :])

                # L5 (out = m2 @ C + bC) is emitted one tile LATE: the PE
                # executes in order, so emitting L5 here would make the next
                # tile's L1 wait behind L5's dependency on this tile's
                # gelu->add chain. Deferred one tile, its inputs are long
                # ready and the PE never stalls.
                if prev is not None:
                    emit_l5(*prev)
                prev = (m2s_t, bass.ts(t, TILE_N), t)

            emit_l5(*prev)

    nc.compile()
    return nc


_NC_CACHE = None


def _run(inputs: dict, trace: bool = False):
    global _NC_CACHE
    if _NC_CACHE is None:
        _NC_CACHE = _build_nc()
    nc = _NC_CACHE

    x = np.ascontiguousarray(inputs["x"], dtype=np.float32)
    consts = _prep_constants(
        *(np.asarray(inputs[k], dtype=np.float32)
          for k in ("W1", "b1", "W2", "b2", "adj1", "Wg1", "bg1",
                    "adj2", "Wg2", "bg2", "Wc", "bc"))
    )

    xflat = x.reshape(ROWS, D)
    in_maps = []
    for i in range(N_CORES):
        shard = np.ascontiguousarray(xflat[i * R_CORE : (i + 1) * R_CORE].T)
        m = {"xT": shard}
        m.update(consts)
        in_maps.append(m)

    res = run_bass_kernel_spmd(nc, in_maps, core_ids=list(range(N_CORES)), trace=trace)
    parts = [np.asarray(r["outT"]).T for r in res.results]     # each [8192, 34]
    out = np.concatenate(parts, axis=0).reshape(B, W, NN, 2)
    return np.ascontiguousarray(out, dtype=np.float32), res


def kernel(**inputs) -> np.ndarray:
    out, _ = _run(inputs, trace=False)
    return out

